# revision 3
# baseline (speedup 1.0000x reference)
"""Trainium2 Bass kernel for the 4-group sparse-tap 3x3 conv.

Computation (see reference): x (32,128,56,56) f32, weights (32,2048) f32.
Four groups of 32 output channels; group g uses 4 taps CFG[g] of the 3x3
footprint over all 128 input channels. Output (32,128,56,56) f32.

Strategy: pure data-parallel over batch — 4 images per NeuronCore, 8 cores.
On host: zero-pad each image to 59x58 (1-pixel conv halo + one extra row so
the last shifted matmul view stays in-bounds) and expand the weights into 9
dense 128x128 stationary matrices (one per tap position, zeros for groups
that don't use the tap).

On device, per image: DMA the padded image into SBUF (128 input channels on
partitions, 59*58=3422 f32 free).  For each chunk of 8 output rows, issue 9
accumulating matmuls into one PSUM bank — tap (kr,kc) uses the rhs slice
starting at (8c+kr)*58+kc, which yields all 8 shifted rows in one contiguous
view thanks to the width padding.  Matmuls run as float32r (FP22 multiply,
FP32 accumulate) which streams 1 column/cycle on the PE at N>=256, 4x faster
than true fp32.  DVE copies PSUM -> SBUF dropping the 2 junk columns per row,
and the scalar engine DMAs each finished image back to HBM.
"""

from contextlib import ExitStack

import numpy as np

import concourse.bass as bass
import concourse.mybir as mybir
from concourse.bass_utils import run_bass_kernel_spmd

CFG = [[1, 2, 4, 5], [2, 3, 5, 6], [4, 5, 7, 8], [5, 6, 8, 9]]

B, C, H, W = 32, 128, 56, 56
NCORES = 8
BPC = B // NCORES            # images per core
HP, WP = H + 3, W + 2        # padded rows (1 top, 1 bottom, 1 overread), cols
XF = HP * WP                 # 3422 padded free elems per image
OF = H * W                   # 3136 output free elems per image
NTAP = 9
NPSUM = 8                    # psum banks cycled over chunks
RPC = 8                      # output rows per chunk
NCHUNK = H // RPC            # 7
NFREE = RPC * WP             # 464 matmul free dim
F32 = mybir.dt.float32
F32R = mybir.dt.float32r


def _build_nc():
    nc = bass.Bass()
    xp = nc.declare_dram_parameter("x", [BPC, C, XF], F32R, isOutput=False)
    wp = nc.declare_dram_parameter("w", [C, NTAP * C], F32R, isOutput=False)
    op = nc.declare_dram_parameter("out", [BPC, C, OF], F32, isOutput=True)

    with ExitStack() as ctx:
        w_tile = ctx.enter_context(nc.sbuf_tensor("w_tile", [C, NTAP * C], F32R))
        x_slots = [ctx.enter_context(nc.sbuf_tensor(f"x_slot{i}", [C, XF], F32R)) for i in range(2)]
        o_slots = [ctx.enter_context(nc.sbuf_tensor(f"o_slot{i}", [C, OF], F32)) for i in range(2)]
        psums = [ctx.enter_context(nc.psum_tensor(f"psum{i}", [C, 512], F32)) for i in range(NPSUM)]

        w_sem = ctx.enter_context(nc.semaphore("w_sem"))
        x_sem = ctx.enter_context(nc.semaphore("x_sem"))
        mm_sem = ctx.enter_context(nc.semaphore("mm_sem"))
        v_sem = ctx.enter_context(nc.semaphore("v_sem"))
        o_sem = ctx.enter_context(nc.semaphore("o_sem"))

        block = ctx.enter_context(nc.Block())

        @block.sync
        def _(sync):
            sync.dma_start(out=w_tile[:], in_=wp[:]).then_inc(w_sem, 16)
            for b in range(BPC):
                if b >= 2:
                    # x slot b%2 is free once image b-2's matmuls are done
                    sync.wait_ge(mm_sem, NCHUNK * (b - 1))
                sync.dma_start(out=x_slots[b % 2][:], in_=xp[b]).then_inc(x_sem, 16)

        @block.tensor
        def _(tensor):
            tensor.wait_ge(w_sem, 16)
            for b in range(BPC):
                tensor.wait_ge(x_sem, 16 * (b + 1))
                for c in range(NCHUNK):
                    g = NCHUNK * b + c
                    if g >= NPSUM:
                        # psum bank g%NPSUM free once chunk g-NPSUM was copied
                        tensor.wait_ge(v_sem, g - NPSUM + 1)
                    ps = psums[g % NPSUM]
                    for u in range(NTAP):
                        kr, kc = u // 3, u % 3
                        off = (RPC * c + kr) * WP + kc
                        mm = tensor.matmul(
                            ps[:, :NFREE],
                            w_tile[:, u * C:(u + 1) * C],
                            x_slots[b % 2][:, off:off + NFREE],
                            start=(u == 0),
                            stop=(u == NTAP - 1),
                        )
                        if u == NTAP - 1:
                            mm.then_inc(mm_sem, 1)

        @block.vector
        def _(vector):
            for b in range(BPC):
                if b >= 2:
                    # out slot b%2 free once image b-2's store DMA completed
                    vector.wait_ge(o_sem, 16 * (b - 1))
                for c in range(NCHUNK):
                    g = NCHUNK * b + c
                    vector.wait_ge(mm_sem, g + 1)
                    src = psums[g % NPSUM][:, :NFREE].rearrange(
                        "p (r w) -> p r w", w=WP)[:, :, :W]
                    dst = o_slots[b % 2][:, c * RPC * W:(c + 1) * RPC * W].rearrange(
                        "p (r w) -> p r w", w=W)
                    vector.tensor_copy(out=dst, in_=src).then_inc(v_sem, 1)

        @block.scalar
        def _(scalar):
            for b in range(BPC):
                scalar.wait_ge(v_sem, NCHUNK * (b + 1))
                scalar.dma_start(out=op[b], in_=o_slots[b % 2][:]).then_inc(o_sem, 16)
            scalar.wait_ge(o_sem, 16 * BPC)

    return nc


_NC_CACHE = None


def _get_nc():
    global _NC_CACHE
    if _NC_CACHE is None:
        _NC_CACHE = _build_nc()
    return _NC_CACHE


def _prep_weights(weights):
    """(32, 2048) grouped-sparse -> 9 dense [ic=128, oc=128] lhsT matrices."""
    w9 = np.zeros((C, NTAP, C), np.float32)
    for g, taps in enumerate(CFG):
        blk = np.asarray(weights[:, g * 512:(g + 1) * 512], np.float32)
        blk = blk.reshape(32, C, 4)  # [oc_in_group, ic, tap_j]
        for j, t in enumerate(taps):
            w9[:, t - 1, 32 * g:32 * (g + 1)] = blk[:, :, j].T
    return np.ascontiguousarray(w9.reshape(C, NTAP * C))


def kernel(x, weights):
    x = np.asarray(x, np.float32)
    weights = np.asarray(weights, np.float32)

    xpad = np.zeros((B, C, HP, WP), np.float32)
    xpad[:, :, 1:H + 1, 1:W + 1] = x
    xs = xpad.reshape(B, C, XF)
    wflat = _prep_weights(weights)

    nc = _get_nc()
    in_maps = [
        {"x": np.ascontiguousarray(xs[i * BPC:(i + 1) * BPC]), "w": wflat}
        for i in range(NCORES)
    ]
    res = run_bass_kernel_spmd(nc, in_maps, core_ids=list(range(NCORES)))
    return np.concatenate(
        [res.results[i]["out"].reshape(BPC, C, H, W) for i in range(NCORES)],
        axis=0,
    )


# revision 4
# speedup vs baseline: 1.8256x; 1.8256x over previous
"""Trainium2 Bass kernel for the 4-group sparse-tap 3x3 conv.

Computation (see reference): x (32,128,56,56) f32, weights (32,2048) f32.
Four groups of 32 output channels; group g uses 4 taps CFG[g] of the 3x3
footprint over all 128 input channels. Output (32,128,56,56) f32.

Strategy: pure data-parallel over batch — 4 images per NeuronCore, 8 cores.

Host prep: zero-pad each image to 59x58 (1-pixel conv halo + one extra row
so the last shifted matmul view stays in-bounds), cast to fp16, and lay the
4 images of a core out channel-major ([128 ic, 4*59*58]) so the whole shard
DMAs with one 27KB-contiguous descriptor per partition.  Weights are
rearranged into 16 [ic=128, oc=32] fp16 stationary blocks, one per
(group, tap) pair.

Device, per image: for each chunk of 8 output rows, issue 16 column-tiled
matmuls (tile_position=(0,32g)) — group g's 4 taps accumulate into PSUM
partitions 32g..32g+31.  Tap (kr,kc) uses the rhs slice starting at
(8c+kr)*58+kc, which yields all 8 shifted rows in one contiguous view
thanks to the width padding.  The 4 groups' matmuls execute concurrently
on the PE's 32-column sub-arrays, so a chunk costs ~4 matmul streams
instead of 9 (the dense-tap formulation).  fp16 keeps 10 mantissa bits
(measured end-to-end error ~5e-4) and accumulates in fp32 PSUM.

DMA plan (descriptor generation is the scarce resource on the HWDGE rings,
~64ns/descriptor; SWDGE generates in parallel): weights + all input images
go on the gpsimd/SWDGE queue; outputs alternate between the sync and
scalar HWDGE rings; the last image's output is split so only a small piece
remains after the final matmul.
"""

from contextlib import ExitStack

import numpy as np

import concourse.bass as bass
import concourse.mybir as mybir
from concourse.bass_utils import run_bass_kernel_spmd

CFG = [[1, 2, 4, 5], [2, 3, 5, 6], [4, 5, 7, 8], [5, 6, 8, 9]]

B, C, H, W = 32, 128, 56, 56
NCORES = 8
BPC = B // NCORES            # images per core
HP, WP = H + 3, W + 2        # padded rows (1 top, 1 bottom, 1 overread), cols
XF = HP * WP                 # 3422 padded free elems per image
OF = H * W                   # 3136 output free elems per image
NPSUM = 8                    # psum banks cycled over chunks
RPC = 8                      # output rows per chunk
NCHUNK = H // RPC            # 7
NFREE = RPC * WP             # 464 matmul free dim
F32 = mybir.dt.float32
F16 = mybir.dt.float16


def _build_nc():
    nc = bass.Bass()
    xp = nc.declare_dram_parameter("x", [C, BPC * XF], F16, isOutput=False)
    wp = nc.declare_dram_parameter("w", [C, 16 * 32], F16, isOutput=False)
    op = nc.declare_dram_parameter("out", [BPC, C, OF], F32, isOutput=True)

    with ExitStack() as ctx:
        w_tile = ctx.enter_context(nc.sbuf_tensor("w_tile", [C, 16 * 32], F16))
        xbuf = ctx.enter_context(nc.sbuf_tensor("xbuf", [C, BPC * XF], F16))
        o_slots = [ctx.enter_context(nc.sbuf_tensor(f"o_slot{i}", [C, OF], F32))
                   for i in range(2)]
        psums = [ctx.enter_context(nc.psum_tensor(f"psum{i}", [C, 512], F32))
                 for i in range(NPSUM)]

        w_sem = ctx.enter_context(nc.semaphore("w_sem"))
        x0_sem = ctx.enter_context(nc.semaphore("x0_sem"))
        x123_sem = ctx.enter_context(nc.semaphore("x123_sem"))
        mm_sem = ctx.enter_context(nc.semaphore("mm_sem"))
        v_sem = ctx.enter_context(nc.semaphore("v_sem"))
        slot0_sem = ctx.enter_context(nc.semaphore("slot0_sem"))
        slot1_sem = ctx.enter_context(nc.semaphore("slot1_sem"))

        block = ctx.enter_context(nc.Block())

        @block.gpsimd
        def _(gpsimd):
            # SWDGE queue: parallel descriptor generation -> fastest way to
            # bring the weights + first image in.
            gpsimd.dma_start(out=w_tile[:], in_=wp[:]).then_inc(w_sem, 16)
            gpsimd.dma_start(out=xbuf[:, 0:XF], in_=xp[:, 0:XF]).then_inc(x0_sem, 16)
            gpsimd.dma_start(out=xbuf[:, XF:], in_=xp[:, XF:]).then_inc(x123_sem, 16)
            # tail piece: last 3 chunks of image 3
            gpsimd.wait_ge(v_sem, 4 * NCHUNK)
            gpsimd.dma_start(
                out=op[3][:, 4 * RPC * W:],
                in_=o_slots[1][:, 4 * RPC * W:],
            ).then_inc(slot1_sem, 16)
            gpsimd.wait_ge(slot1_sem, 48)

        @block.tensor
        def _(tensor):
            tensor.wait_ge(w_sem, 16)
            tensor.wait_ge(x0_sem, 16)
            for b in range(BPC):
                if b == 1:
                    tensor.wait_ge(x123_sem, 16)
                for c in range(NCHUNK):
                    g = NCHUNK * b + c
                    if g >= NPSUM:
                        # psum bank g%NPSUM free once chunk g-NPSUM was copied
                        tensor.wait_ge(v_sem, g - NPSUM + 1)
                    bank = psums[g % NPSUM]
                    for j in range(4):
                        for grp in range(4):
                            t = CFG[grp][j]
                            kr, kc = (t - 1) // 3, (t - 1) % 3
                            off = b * XF + (RPC * c + kr) * WP + kc
                            idx = grp * 4 + j
                            mm = tensor.matmul(
                                bank[32 * grp:32 * (grp + 1), :NFREE],
                                w_tile[:, idx * 32:(idx + 1) * 32],
                                xbuf[:, off:off + NFREE],
                                start=(j == 0),
                                stop=(j == 3),
                                tile_position=(0, 32 * grp),
                            )
                    mm.then_inc(mm_sem, 1)

        @block.vector
        def _(vector):
            for b in range(BPC):
                if b == 2:
                    vector.wait_ge(slot0_sem, 16)   # out0 done -> slot0 free
                if b == 3:
                    vector.wait_ge(slot1_sem, 16)   # out1 done -> slot1 free
                for c in range(NCHUNK):
                    g = NCHUNK * b + c
                    vector.wait_ge(mm_sem, g + 1)
                    src = psums[g % NPSUM][:, :NFREE].rearrange(
                        "p (r w) -> p r w", w=WP)[:, :, :W]
                    dst = o_slots[b % 2][:, c * RPC * W:(c + 1) * RPC * W].rearrange(
                        "p (r w) -> p r w", w=W)
                    vector.tensor_copy(out=dst, in_=src).then_inc(v_sem, 1)

        @block.sync
        def _(sync):
            sync.wait_ge(v_sem, NCHUNK)
            sync.dma_start(out=op[0], in_=o_slots[0][:]).then_inc(slot0_sem, 16)
            sync.wait_ge(v_sem, 3 * NCHUNK)
            sync.dma_start(out=op[2], in_=o_slots[0][:]).then_inc(slot0_sem, 16)
            sync.wait_ge(slot0_sem, 32)

        @block.scalar
        def _(scalar):
            scalar.wait_ge(v_sem, 2 * NCHUNK)
            scalar.dma_start(out=op[1], in_=o_slots[1][:]).then_inc(slot1_sem, 16)
            # first 4 chunks of image 3, issued as soon as they are copied
            scalar.wait_ge(v_sem, 3 * NCHUNK + 4)
            scalar.dma_start(
                out=op[3][:, :4 * RPC * W],
                in_=o_slots[1][:, :4 * RPC * W],
            ).then_inc(slot1_sem, 16)
            scalar.wait_ge(slot1_sem, 48)

    return nc


_NC_CACHE = None


def _get_nc():
    global _NC_CACHE
    if _NC_CACHE is None:
        _NC_CACHE = _build_nc()
    return _NC_CACHE


def _prep_weights(weights):
    """(32, 2048) grouped-sparse -> 16 [ic=128, oc=32] fp16 lhsT blocks."""
    w16 = np.zeros((C, 16 * 32), np.float32)
    for g, taps in enumerate(CFG):
        blk = np.asarray(weights[:, g * 512:(g + 1) * 512], np.float32)
        blk = blk.reshape(32, C, 4)  # [oc_in_group, ic, tap_j]
        for j in range(4):
            idx = g * 4 + j
            w16[:, idx * 32:(idx + 1) * 32] = blk[:, :, j].T
    return np.ascontiguousarray(w16.astype(np.float16))


def _prep_x(x):
    """(32,128,56,56) f32 -> per-core channel-major padded fp16 shards."""
    xpad = np.zeros((B, C, HP, WP), np.float16)
    xpad[:, :, 1:H + 1, 1:W + 1] = x.astype(np.float16)
    xs = xpad.reshape(NCORES, BPC, C, XF)
    # (core, b, c, f) -> (core, c, b*f)
    xs = np.ascontiguousarray(xs.transpose(0, 2, 1, 3)).reshape(NCORES, C, BPC * XF)
    return xs


def kernel(x, weights):
    x = np.asarray(x, np.float32)
    weights = np.asarray(weights, np.float32)

    xs = _prep_x(x)
    wflat = _prep_weights(weights)

    nc = _get_nc()
    in_maps = [{"x": xs[i], "w": wflat} for i in range(NCORES)]
    res = run_bass_kernel_spmd(nc, in_maps, core_ids=list(range(NCORES)))
    return np.concatenate(
        [res.results[i]["out"].reshape(BPC, C, H, W) for i in range(NCORES)],
        axis=0,
    )
